# revision 38
# baseline (speedup 1.0000x reference)
"""Trainium2 Bass kernel for nn_CoordinateDescent (B=2, M=N=2048, R=16).

Math: the coordinate-descent residual e never needs materializing. With
G = v^T v and c = x @ v, the per-rank recurrence collapses to a 16x16
triangular solve per row:  a @ L = y,  L = (D+eps) + strict_upper(G),
y = c + eps - u @ strict_lower(G);  transposed  aT = W' cT' with
W' = (I+Z)^-1 diag(rd), Z = rd .* strict_lower(G), rd = 1/(diag(G)+eps),
cT' = cT - strict_lower(G)^T u^T.  (The +eps terms inside y contribute
~1e-8 absolute to a — far below fp8 rounding — and are folded into the
host-side f64 finishing step instead.)

Sharding: 8 cores = batch (2) x M-chunk (4). Core (b, j) owns 512 rows of
x[b]: it computes cT for its chunk, applies the solve to get u_new[ms_j]
on device, then computes the phase-2 partial ct2_j = u_new[ms_j]^T @
x[ms_j, :] over ALL n. The cross-chunk combine is linear, so the host sums
the 4 partials per batch and finishes the tiny 16x16 phase-2 triangular
solve in f64. No collectives (ncfw rendezvous dead time ~100us).

Numerics: x, v, and the phase-2 u_new operand ride in fp8 e4m3 (measured
end-to-end rel err: u ~4.4e-3, v ~6.9e-3 vs the 2e-2 gate; dot-product
quantization noise averages out over the 2048-long contraction). This
halves HBM traffic vs bf16 AND halves matmul count via
MatmulPerfMode.DoubleRow (two 128-deep k-subtiles per pass; warm
pipelined rate measured 216ns per 512-col pair). The 16x16 solve runs in
bf16 (f32 matmuls lower to 2 LDWEIGHTS+2 MATMUL passes — measured 1.9us
vs 1.0us for bf16). u_new is scaled by 16 before the fp8 downcast so
phase-2 operands sit in e4m3's normal range; the host divides ct2 back.
ct2 partials leave as bf16 (8-bit mantissa, v err unchanged).

Schedule: all DMAs ride the sync hardware-DGE queue in FIFO order (each
dma_start costs ~625ns of SP issue time and the completion semaphore
propagates ~900ns after the transfer). DMA descriptor size = the
contiguous per-partition run, and per-descriptor engine overhead
(~165ns) dominates small descriptors: 2KB descriptors measured only
~150GB/s/core vs ~300+ at 8KB, and ANY extra 128-descriptor dma_start
costs ~1us of stream time. So xv streams as ONE dma (8448B/partition)
and the x-row copy as a 3/4+1/4 pair, so most phase-2 matmuls run
before the final quarter lands. The PE clock is HAM-gated (1.2 GHz
cold, 2.4 GHz after a ~3.4us sustained-busy window, re-throttling after
a mostly-idle window): dummy matmuls on scratch data fill the stream
lead-in and every inter-phase gap, which halves all matmul times and is
worth ~3us end-to-end. Phase-2 writes five psum chunks, the last a
128-col stub whose tiny copy+DMA forms the final off-stream chain;
psum->SBUF copies split across Vector+Scalar (PSUM reads on 16
partitions are the tail bottleneck). Kernel is HBM-stream-bound:
~2.2MB/core at the 8-core shared-HBM aggregate (~2.3TB/s) plus ~9.4us
of fixed NRT postamble/exit (sync_barrier + 51-sem reset per engine +
dma_rearm) and ~1.3us of in-window preamble.
"""

import os
import numpy as np
import ml_dtypes

import concourse.bass as bass
import concourse.mybir as mybir
import concourse.tile as tile
from concourse import bacc
from concourse.bass_utils import run_bass_kernel_spmd

B, M, N, R = 2, 2048, 2048, 16
NCORES = 8
NJ = 4            # m-chunks per batch
SH = M // NJ      # 512
P = 128
KO = N // P       # 16 n-tiles of 128 (phase-1 contraction)
KO2 = KO // 2     # 8 DoubleRow pair groups
MT = SH // P      # 4 m-tiles of 128 in a chunk
NQ = 4            # 512-wide n-quarters for phase-2 psum
NC = N // NQ      # 512
UNSCALE = 16.0    # u_new fp8 pre-scale (max|u_new*16| << 240)
EPS = 1e-8

F32 = mybir.dt.float32
BF16 = mybir.dt.bfloat16
F8 = mybir.dt.float8e4
ALU = mybir.AluOpType
DR = mybir.MatmulPerfMode.DoubleRow

_CACHE = {}


def _build_nc():
    nc = bacc.Bacc(
        "TRN2",
        target_bir_lowering=False,
        debug=False,
        num_devices=NCORES,
    )

    # xt and vb interleave per ko-tile in one tensor: cols 0:SH are the x^T
    # block, SH:SH+R the matching v tile — one DMA stream, no handoff gap
    xv_d = nc.dram_tensor("xv", [P, KO, SH + R], F8, kind="ExternalInput")
    # q-major layout: each n-quarter chunk is 2KB contiguous per partition
    xm_d = nc.dram_tensor("xm", [P, NQ, MT, NC], F8, kind="ExternalInput")
    ws_d = nc.dram_tensor("ws", [R, SH + R], F32, kind="ExternalInput")    # su | W'^T
    ou_d = nc.dram_tensor("ou", [P, MT, R], F32, kind="ExternalOutput")    # u_new chunk
    oc_d = nc.dram_tensor("oc", [R, N], BF16, kind="ExternalOutput")       # ct2 partial

    with tile.TileContext(nc, num_cores=NCORES) as tc:
        with (
            tc.tile_pool(name="big", bufs=1) as big,
            tc.tile_pool(name="cst", bufs=1) as cst,
            tc.tile_pool(name="cps", bufs=1, space="PSUM") as cps,
            tc.tile_pool(name="ops", bufs=1, space="PSUM") as ops,
            tc.tile_pool(name="c2p", bufs=1, space="PSUM") as c2p,
        ):
            ws = cst.tile([R, SH + R], F32, tag="ws")
            wzb = cst.tile([R, R], BF16, tag="wzb")
            yb = cst.tile([R, SH], BF16, tag="yb")
            un = cst.tile([P, MT, R], F8, tag="un")
            ounat = cst.tile([P, MT, R], F32, tag="ounat")
            oc = cst.tile([R, N], BF16, tag="oc")
            ds = cst.tile([P, NC], BF16, tag="ds")      # warm-up scratch

            xv = big.tile([P, KO, SH + R], F8, tag="xv")
            xm = big.tile([P, NQ, MT, NC], F8, tag="xm")

            # The PE clock is HAM-gated: 1.2 GHz cold, 2.4 GHz after ~3.4us of
            # sustained busy (free-running 4096-cycle window). Real matmuls
            # here are DMA-paced with idle gaps, so without help the PE never
            # warms and every matmul runs at half speed. Dummy matmuls on
            # scratch data fill the stream lead-in and inter-phase gaps to
            # assert and hold the busy window. They sit in the in-order PE
            # queue, so counts are tuned to the measured gap lengths.
            # DVE memset: unblocks the burst ~0.5us earlier than gpsimd's.
            nc.vector.memset(ds[:], 0.0)
            dp = c2p.tile([R, NC], F32, tag="dp", name="dp")

            def warm(k, n=NC):
                for _ in range(k):
                    nc.tensor.matmul(dp[:, 0:n], ds[:, 0:R], ds[:, 0:n])

            # xv leads the stream as one DMA (8448B/partition descriptors —
            # bigger descriptors beat finer pacing: a 2-chunk split measured
            # ~2us slower end-to-end); ws rides second (needed at solve time).
            nc.sync.dma_start(xv[:], xv_d[:])
            nc.sync.dma_start(ws[:], ws_d[:])
            nc.sync.dma_start(xm[:], xm_d[:])
            # downcast W'^T once for the bf16 solve; depends only on ws
            nc.vector.tensor_copy(out=wzb[:], in_=ws[:, SH : SH + R])

            # warm-up burst: >=3.4us of back-to-back PE work during the DMA
            # stream lead-in flips HAM to 8/8 before the first real matmul;
            # the trailing short fills keep the free-running window
            # saturated until the xv data lands. Once warm, only a ~3.4us
            # idle window re-throttles, so small inter-phase gaps are safe.
            warm(6)
            warm(14, 128)

            # ---- phase 1: cT for the m-chunk, then the collapsed solve ----
            # 8 DoubleRow matmuls, each contracting two 128-deep n-tiles.
            ct = cps.tile([R, SH], F32, tag="cps")
            for j in range(KO2):
                nc.tensor.matmul(
                    ct[:],
                    xv[:, 2 * j : 2 * j + 2, SH : SH + R],
                    xv[:, 2 * j : 2 * j + 2, 0:SH],
                    start=(j == 0), stop=(j == KO2 - 1),
                    perf_mode=DR,
                )
            warm(3, 128)           # fill the TT0 window

            # The solve chain is split by m-halves so phase 2 starts as soon
            # as the first two u_new tiles exist: TT half -> 2 solve matmuls
            # -> fp8 downcast -> the half's phase-2 (start) matmuls, with the
            # second half's solve interleaved into the PE queue behind them.
            HS = SH // 2
            nc.vector.tensor_tensor(yb[:, 0:HS], ct[:, 0:HS], ws[:, 0:HS], ALU.add)
            nc.vector.tensor_tensor(yb[:, HS:SH], ct[:, HS:SH], ws[:, HS:SH], ALU.add)
            oup = ops.tile([P, MT * R], F32, tag="oup")
            c2 = [
                c2p.tile([R, NC], F32, tag=f"c2_{q}", name=f"c2_{q}")
                for q in range(NQ)
            ]
            # phase-2 n-chunks (q index, col offset in q, width): four full
            # psum banks, then a 128-col stub whose tiny copy+DMA forms the
            # final off-stream chain. The stub shares q3's data but its own
            # psum tile (in the ops pool next to oup).
            stub = ops.tile([R, P], F32, tag="stub")
            CH2 = [
                (c2[0], 0, 0, NC), (c2[1], 1, 0, NC), (c2[2], 2, 0, NC),
                (c2[3], 3, 0, NC - P), (stub, 3, NC - P, P),
            ]

            def p2mm(i, t2, start, stop):
                tile_, q, qo, w = CH2[i]
                nc.tensor.matmul(
                    tile_[:, 0:w],
                    un[:, 2 * t2 : 2 * t2 + 2, :],
                    xm[:, q, 2 * t2 : 2 * t2 + 2, qo : qo + w],
                    start=start, stop=stop, perf_mode=DR,
                )

            for t in (0, 1):
                nc.tensor.matmul(
                    oup[:, t * R : (t + 1) * R], yb[:, t * P : (t + 1) * P], wzb[:]
                )
            nc.scalar.activation(
                un[:, 0:2].rearrange("p t r -> p (t r)"), oup[:, 0 : 2 * R],
                mybir.ActivationFunctionType.Copy,
                scale=UNSCALE,
            )
            warm(4, 128)           # fill the un downcast window
            for i in (0, 1):
                p2mm(i, 0, True, False)
            warm(2, 128)           # keep the HAM window busy through the
            for t in (2, 3):
                nc.tensor.matmul(
                    oup[:, t * R : (t + 1) * R], yb[:, t * P : (t + 1) * P], wzb[:]
                )
            nc.scalar.activation(
                un[:, 2:4].rearrange("p t r -> p (t r)"), oup[:, 2 * R : 4 * R],
                mybir.ActivationFunctionType.Copy,
                scale=UNSCALE,
            )
            warm(3, 128)           # solve/downcast chain (a mid-kernel
            p2mm(2, 0, True, False)  # re-throttle costs ~2-3us); chunk-2's
            # open matmul covers the ACT1->PE semaphore latency
            nc.vector.tensor_copy(
                out=ounat[:].rearrange("p t r -> p (t r)"), in_=oup[:]
            )
            nc.sync.dma_start(ou_d[:], ounat[:])

            # ---- phase 2 close: each chunk closes as early as possible so
            # its psum->SBUF copy starts draining immediately (copies are
            # PSUM-read bound on 16 partitions and the two copy engines are
            # the tail bottleneck; splitting each chunk across Vector+Scalar
            # halves the latency; bf16 out halves the output DMA bytes —
            # partials are 8-bit-mantissa safe, measured v err unchanged).
            def close_and_copy(i):
                tile_, q, qo, w = CH2[i]
                o = q * NC + qo
                p2mm(i, 1, False, True)
                if w > P:
                    h = w // 2
                    nc.vector.tensor_copy(
                        out=oc[:, o : o + h], in_=tile_[:, 0:h]
                    )
                    nc.scalar.activation(
                        oc[:, o + h : o + w], tile_[:, h:w],
                        mybir.ActivationFunctionType.Copy,
                    )
                else:
                    nc.vector.tensor_copy(out=oc[:, o : o + w], in_=tile_[:, 0:w])

            close_and_copy(0)
            close_and_copy(1)
            nc.sync.dma_start(oc_d[:, 0 : N // 2], oc[:, 0 : N // 2])
            close_and_copy(2)
            p2mm(3, 0, True, False)
            close_and_copy(3)
            p2mm(4, 0, True, False)
            close_and_copy(4)
            nc.sync.dma_start(oc_d[:, N // 2 : N], oc[:, N // 2 : N])

    nc.compile()
    return nc


def _host_solver_inputs(v):
    """Per batch: -strict_lower(G) and W'^T, W' = inv(I+Z) diag(rd), G = v^T v."""
    out = []
    for b in range(B):
        vb = np.asarray(v[b], np.float64)
        G = vb.T @ vb
        rd = 1.0 / (np.diag(G) + EPS)
        sun = -np.tril(G, -1)
        Z = rd[:, None] * np.tril(G, -1)
        W = np.linalg.inv(np.eye(R) + Z) * rd[None, :]  # (I+Z)^-1 then col-scale
        out.append((sun, np.ascontiguousarray(W.T).astype(np.float32)))
    return out


def _prep_in_maps(x, u, v):
    f8 = ml_dtypes.float8_e4m3
    solver = _host_solver_inputs(v)
    per_batch = []
    for b in range(B):
        xq = np.asarray(x[b], np.float32).astype(f8)      # quantize once
        vq = np.asarray(v[b], np.float32).astype(f8)
        vbb = vq.reshape(KO, P, R).swapaxes(0, 1)         # [P, KO, R]
        per_batch.append((xq, vbb))
    in_maps = []
    for c in range(NCORES):
        b, j = divmod(c, NJ)
        xq, vbb = per_batch[b]
        sun, wzt = solver[b]
        ms = slice(j * SH, (j + 1) * SH)
        xc = xq[ms]  # [SH, N] fp8
        xt = np.ascontiguousarray(xc.T).reshape(KO, P, SH).swapaxes(0, 1)
        xvv = np.concatenate([xt, vbb], axis=2)           # [P, KO, SH+R] fp8
        # [P, NQ, MT, NC]: q-major so each chunk is contiguous/partition
        xmm = np.ascontiguousarray(
            xc.reshape(MT, P, NQ, NC).transpose(1, 2, 0, 3)
        )
        # su = -SL(G)^T @ u[ms]^T : the u-correction to cT, [R, SH] f32
        su = (
            sun.T @ np.asarray(u[b], np.float64)[ms].T
        ).astype(np.float32)
        ws = np.concatenate([su, wzt], axis=1)            # [R, SH+R] f32
        in_maps.append({"xv": xvv, "xm": xmm, "ws": ws})
    return in_maps


def _host_phase2(u_new, ct2, v):
    """Finish the v update: tiny 16x16 triangular solve per batch, f64.

    G2 uses the same fp8-quantized u_new the device used for ct2, so the
    phase-2 solve is exact coordinate descent w.r.t. that u.
    """
    f8 = ml_dtypes.float8_e4m3
    v_new = np.empty((B, N, R), np.float32)
    for b in range(B):
        un = (
            (u_new[b] * np.float32(UNSCALE)).astype(f8).astype(np.float64)
            / UNSCALE
        )
        G2 = un.T @ un
        c2 = ct2[b].T.astype(np.float64) / UNSCALE        # [N, R] = x^T @ u_new
        Y2 = c2 + EPS - np.asarray(v[b], np.float64) @ np.tril(G2, -1)
        L2 = np.triu(G2, 1) + np.diag(np.diag(G2) + EPS)
        v_new[b] = np.linalg.solve(L2.T, Y2.T).T.astype(np.float32)
    return v_new


def run(x, u, v, trace=False, trace_cores=None):
    if "nc" not in _CACHE:
        _CACHE["nc"] = _build_nc()
    nc = _CACHE["nc"]
    in_maps = _prep_in_maps(x, u, v)
    kw = {}
    if trace_cores is not None:
        kw["trace_cores"] = trace_cores
    res = run_bass_kernel_spmd(
        nc, in_maps, core_ids=list(range(NCORES)), trace=trace, **kw
    )
    u_new = np.empty((B, M, R), np.float32)
    ct2 = np.zeros((B, R, N), np.float64)
    for c in range(NCORES):
        b, j = divmod(c, NJ)
        ms = slice(j * SH, (j + 1) * SH)
        u_new[b, ms] = (
            np.asarray(res.results[c]["ou"]).transpose(1, 0, 2).reshape(SH, R)
        )
        ct2[b] += np.asarray(res.results[c]["oc"])
    v_new = _host_phase2(u_new, ct2, v)
    return (u_new, v_new), res


def kernel(x, u, v):
    (u_new, v_new), _ = run(x, u, v, trace=bool(os.environ.get("CD_TRACE")))
    return (u_new, v_new)


# revision 40
# speedup vs baseline: 1.0066x; 1.0066x over previous
"""Trainium2 Bass kernel for nn_CoordinateDescent (B=2, M=N=2048, R=16).

Math: the coordinate-descent residual e never needs materializing. With
G = v^T v and c = x @ v, the per-rank recurrence collapses to a 16x16
triangular solve per row:  a @ L = y,  L = (D+eps) + strict_upper(G),
y = c + eps - u @ strict_lower(G);  transposed  aT = W' cT' with
W' = (I+Z)^-1 diag(rd), Z = rd .* strict_lower(G), rd = 1/(diag(G)+eps),
cT' = cT - strict_lower(G)^T u^T.  (The +eps terms inside y contribute
~1e-8 absolute to a — far below fp8 rounding — and are folded into the
host-side f64 finishing step instead.)

Sharding: 8 cores = batch (2) x M-chunk (4). Core (b, j) owns 512 rows of
x[b]: it computes cT for its chunk, applies the solve to get u_new[ms_j]
on device, then computes the phase-2 partial ct2_j = u_new[ms_j]^T @
x[ms_j, :] over ALL n. The cross-chunk combine is linear, so the host sums
the 4 partials per batch and finishes the tiny 16x16 phase-2 triangular
solve in f64. No collectives (ncfw rendezvous dead time ~100us).

Numerics: x, v, and the phase-2 u_new operand ride in fp8 e4m3 (measured
end-to-end rel err: u ~4.4e-3, v ~6.9e-3 vs the 2e-2 gate; dot-product
quantization noise averages out over the 2048-long contraction). This
halves HBM traffic vs bf16 AND halves matmul count via
MatmulPerfMode.DoubleRow (two 128-deep k-subtiles per pass; warm
pipelined rate measured 216ns per 512-col pair). The 16x16 solve runs in
bf16 (f32 matmuls lower to 2 LDWEIGHTS+2 MATMUL passes — measured 1.9us
vs 1.0us for bf16). u_new is scaled by 16 before the fp8 downcast so
phase-2 operands sit in e4m3's normal range; the host divides ct2 back.
ct2 partials leave as bf16 (8-bit mantissa, v err unchanged).

Schedule: all DMAs ride the sync hardware-DGE queue in FIFO order (each
dma_start costs ~625ns of SP issue time and the completion semaphore
propagates ~900ns after the transfer). DMA descriptor size = the
contiguous per-partition run, and per-descriptor engine overhead
(~165ns) dominates small descriptors: 2KB descriptors measured only
~150GB/s/core vs ~300+ at 8KB, and ANY extra 128-descriptor dma_start
costs ~1us of stream time. So xv and the x-row copy each stream as ONE
dma (8448B / 8192B per-partition descriptors); every finer split
benchmarked slower. The PE clock is HAM-gated (1.2 GHz
cold, 2.4 GHz after a ~3.4us sustained-busy window, re-throttling after
a mostly-idle window): dummy matmuls on scratch data fill the stream
lead-in and every inter-phase gap, which halves all matmul times and is
worth ~3us end-to-end. Phase-2 writes five psum chunks, the last a
128-col stub whose tiny copy+DMA forms the final off-stream chain;
psum->SBUF copies split across Vector+Scalar (PSUM reads on 16
partitions are the tail bottleneck). Kernel is HBM-stream-bound:
~2.2MB/core at the 8-core shared-HBM aggregate (~2.3TB/s) plus ~9.4us
of fixed NRT postamble/exit (sync_barrier + 51-sem reset per engine +
dma_rearm) and ~1.3us of in-window preamble.
"""

import os
import numpy as np
import ml_dtypes

import concourse.bass as bass
import concourse.mybir as mybir
import concourse.tile as tile
from concourse import bacc
from concourse.bass_utils import run_bass_kernel_spmd

B, M, N, R = 2, 2048, 2048, 16
NCORES = 8
NJ = 4            # m-chunks per batch
SH = M // NJ      # 512
P = 128
KO = N // P       # 16 n-tiles of 128 (phase-1 contraction)
KO2 = KO // 2     # 8 DoubleRow pair groups
MT = SH // P      # 4 m-tiles of 128 in a chunk
NQ = 4            # 512-wide n-quarters for phase-2 psum
NC = N // NQ      # 512
UNSCALE = 16.0    # u_new fp8 pre-scale (max|u_new*16| << 240)
EPS = 1e-8

F32 = mybir.dt.float32
BF16 = mybir.dt.bfloat16
F8 = mybir.dt.float8e4
ALU = mybir.AluOpType
DR = mybir.MatmulPerfMode.DoubleRow

_CACHE = {}


def _build_nc():
    nc = bacc.Bacc(
        "TRN2",
        target_bir_lowering=False,
        debug=False,
        num_devices=NCORES,
    )

    # xt and vb interleave per ko-tile in one tensor: cols 0:SH are the x^T
    # block, SH:SH+R the matching v tile — one DMA stream, no handoff gap
    xv_d = nc.dram_tensor("xv", [P, KO, SH + R], F8, kind="ExternalInput")
    # q-major layout: each n-quarter chunk is 2KB contiguous per partition
    xm_d = nc.dram_tensor("xm", [P, NQ, MT, NC], F8, kind="ExternalInput")
    ws_d = nc.dram_tensor("ws", [R, SH + R], F32, kind="ExternalInput")    # su | W'^T
    ou_d = nc.dram_tensor("ou", [P, MT, R], F32, kind="ExternalOutput")    # u_new chunk
    oc_d = nc.dram_tensor("oc", [R, N], BF16, kind="ExternalOutput")       # ct2 partial

    with tile.TileContext(nc, num_cores=NCORES) as tc:
        with (
            tc.tile_pool(name="big", bufs=1) as big,
            tc.tile_pool(name="cst", bufs=1) as cst,
            tc.tile_pool(name="cps", bufs=1, space="PSUM") as cps,
            tc.tile_pool(name="ops", bufs=1, space="PSUM") as ops,
            tc.tile_pool(name="c2p", bufs=1, space="PSUM") as c2p,
        ):
            ws = cst.tile([R, SH + R], F32, tag="ws")
            wzb = cst.tile([R, R], BF16, tag="wzb")
            yb = cst.tile([R, SH], BF16, tag="yb")
            un = cst.tile([P, MT, R], F8, tag="un")
            ounat = cst.tile([P, MT, R], F32, tag="ounat")
            oc = cst.tile([R, N], BF16, tag="oc")
            ds = cst.tile([P, NC], BF16, tag="ds")      # warm-up scratch

            xv = big.tile([P, KO, SH + R], F8, tag="xv")
            xm = big.tile([P, NQ, MT, NC], F8, tag="xm")

            # The PE clock is HAM-gated: 1.2 GHz cold, 2.4 GHz after ~3.4us of
            # sustained busy (free-running 4096-cycle window). Real matmuls
            # here are DMA-paced with idle gaps, so without help the PE never
            # warms and every matmul runs at half speed. Dummy matmuls on
            # scratch data fill the stream lead-in and inter-phase gaps to
            # assert and hold the busy window. They sit in the in-order PE
            # queue, so counts are tuned to the measured gap lengths.
            # DVE memset: unblocks the burst ~0.5us earlier than gpsimd's.
            nc.vector.memset(ds[:], 0.0)
            dp = c2p.tile([R, NC], F32, tag="dp", name="dp")

            def warm(k, n=NC):
                for _ in range(k):
                    nc.tensor.matmul(dp[:, 0:n], ds[:, 0:R], ds[:, 0:n])

            # xv leads the stream as one DMA (8448B/partition descriptors —
            # bigger descriptors beat finer pacing: a 2-chunk split measured
            # ~2us slower end-to-end); ws rides second (needed at solve time).
            nc.sync.dma_start(xv[:], xv_d[:])
            nc.sync.dma_start(ws[:], ws_d[:])
            nc.sync.dma_start(xm[:], xm_d[:])
            # downcast W'^T once for the bf16 solve; depends only on ws
            nc.vector.tensor_copy(out=wzb[:], in_=ws[:, SH : SH + R])

            # warm-up burst: >=3.4us of back-to-back PE work during the DMA
            # stream lead-in flips HAM to 8/8 before the first real matmul;
            # the trailing short fills keep the free-running window
            # saturated until the xv data lands. Once warm, only a ~3.4us
            # idle window re-throttles, so small inter-phase gaps are safe.
            warm(6)
            warm(14, 128)

            # ---- phase 1: cT for the m-chunk, then the collapsed solve ----
            # 8 DoubleRow matmuls, each contracting two 128-deep n-tiles.
            ct = cps.tile([R, SH], F32, tag="cps")
            for j in range(KO2):
                nc.tensor.matmul(
                    ct[:],
                    xv[:, 2 * j : 2 * j + 2, SH : SH + R],
                    xv[:, 2 * j : 2 * j + 2, 0:SH],
                    start=(j == 0), stop=(j == KO2 - 1),
                    perf_mode=DR,
                )
            warm(3, 128)           # fill the TT0 window

            # The solve chain is split by m-halves so phase 2 starts as soon
            # as the first two u_new tiles exist: TT half -> 2 solve matmuls
            # -> fp8 downcast -> the half's phase-2 (start) matmuls, with the
            # second half's solve interleaved into the PE queue behind them.
            HS = SH // 2
            nc.vector.tensor_tensor(yb[:, 0:HS], ct[:, 0:HS], ws[:, 0:HS], ALU.add)
            nc.vector.tensor_tensor(yb[:, HS:SH], ct[:, HS:SH], ws[:, HS:SH], ALU.add)
            oup = ops.tile([P, MT * R], F32, tag="oup")
            c2 = [
                c2p.tile([R, NC], F32, tag=f"c2_{q}", name=f"c2_{q}")
                for q in range(NQ)
            ]
            # phase-2 n-chunks (q index, col offset in q, width): four full
            # psum banks, then a 128-col stub whose tiny copy+DMA forms the
            # final off-stream chain. The stub shares q3's data but its own
            # psum tile (in the ops pool next to oup).
            stub = ops.tile([R, P], F32, tag="stub")
            CH2 = [
                (c2[0], 0, 0, NC), (c2[1], 1, 0, NC), (c2[2], 2, 0, NC),
                (c2[3], 3, 0, NC - P), (stub, 3, NC - P, P),
            ]

            def p2mm(i, t2, start, stop):
                tile_, q, qo, w = CH2[i]
                nc.tensor.matmul(
                    tile_[:, 0:w],
                    un[:, 2 * t2 : 2 * t2 + 2, :],
                    xm[:, q, 2 * t2 : 2 * t2 + 2, qo : qo + w],
                    start=start, stop=stop, perf_mode=DR,
                )

            for t in (0, 1):
                nc.tensor.matmul(
                    oup[:, t * R : (t + 1) * R], yb[:, t * P : (t + 1) * P], wzb[:]
                )
            nc.scalar.activation(
                un[:, 0:2].rearrange("p t r -> p (t r)"), oup[:, 0 : 2 * R],
                mybir.ActivationFunctionType.Copy,
                scale=UNSCALE,
            )
            warm(4, 128)           # fill the un downcast window
            for i in (0, 1):
                p2mm(i, 0, True, False)
            warm(2, 128)           # keep the HAM window busy through the
            for t in (2, 3):
                nc.tensor.matmul(
                    oup[:, t * R : (t + 1) * R], yb[:, t * P : (t + 1) * P], wzb[:]
                )
            nc.scalar.activation(
                un[:, 2:4].rearrange("p t r -> p (t r)"), oup[:, 2 * R : 4 * R],
                mybir.ActivationFunctionType.Copy,
                scale=UNSCALE,
            )
            warm(3, 128)           # solve/downcast chain (a mid-kernel
            p2mm(2, 0, True, False)  # re-throttle costs ~2-3us); chunk-2's
            # open matmul covers the ACT1->PE semaphore latency
            nc.vector.tensor_copy(
                out=ounat[:].rearrange("p t r -> p (t r)"), in_=oup[:]
            )
            nc.sync.dma_start(ou_d[:], ounat[:])

            # ---- phase 2 close: each chunk closes as early as possible so
            # its psum->SBUF copy starts draining immediately (copies are
            # PSUM-read bound on 16 partitions and the two copy engines are
            # the tail bottleneck; splitting each chunk across Vector+Scalar
            # halves the latency; bf16 out halves the output DMA bytes —
            # partials are 8-bit-mantissa safe, measured v err unchanged).
            def close_and_copy(i):
                tile_, q, qo, w = CH2[i]
                o = q * NC + qo
                p2mm(i, 1, False, True)
                if w > P:
                    # V measures ~1.60ns/col, S ~1.78ns/col: split 55/45 so
                    # both copy engines finish together
                    h = (w * 11 // 20) // 8 * 8
                    nc.vector.tensor_copy(
                        out=oc[:, o : o + h], in_=tile_[:, 0:h]
                    )
                    nc.scalar.activation(
                        oc[:, o + h : o + w], tile_[:, h:w],
                        mybir.ActivationFunctionType.Copy,
                    )
                else:
                    nc.vector.tensor_copy(out=oc[:, o : o + w], in_=tile_[:, 0:w])

            close_and_copy(0)
            close_and_copy(1)
            nc.sync.dma_start(oc_d[:, 0 : N // 2], oc[:, 0 : N // 2])
            close_and_copy(2)
            p2mm(3, 0, True, False)
            close_and_copy(3)
            p2mm(4, 0, True, False)
            close_and_copy(4)
            nc.sync.dma_start(oc_d[:, N // 2 : N], oc[:, N // 2 : N])

    nc.compile()
    return nc


def _host_solver_inputs(v):
    """Per batch: -strict_lower(G) and W'^T, W' = inv(I+Z) diag(rd), G = v^T v."""
    out = []
    for b in range(B):
        vb = np.asarray(v[b], np.float64)
        G = vb.T @ vb
        rd = 1.0 / (np.diag(G) + EPS)
        sun = -np.tril(G, -1)
        Z = rd[:, None] * np.tril(G, -1)
        W = np.linalg.inv(np.eye(R) + Z) * rd[None, :]  # (I+Z)^-1 then col-scale
        out.append((sun, np.ascontiguousarray(W.T).astype(np.float32)))
    return out


def _prep_in_maps(x, u, v):
    f8 = ml_dtypes.float8_e4m3
    solver = _host_solver_inputs(v)
    per_batch = []
    for b in range(B):
        xq = np.asarray(x[b], np.float32).astype(f8)      # quantize once
        vq = np.asarray(v[b], np.float32).astype(f8)
        vbb = vq.reshape(KO, P, R).swapaxes(0, 1)         # [P, KO, R]
        per_batch.append((xq, vbb))
    in_maps = []
    for c in range(NCORES):
        b, j = divmod(c, NJ)
        xq, vbb = per_batch[b]
        sun, wzt = solver[b]
        ms = slice(j * SH, (j + 1) * SH)
        xc = xq[ms]  # [SH, N] fp8
        xt = np.ascontiguousarray(xc.T).reshape(KO, P, SH).swapaxes(0, 1)
        xvv = np.concatenate([xt, vbb], axis=2)           # [P, KO, SH+R] fp8
        # [P, NQ, MT, NC]: q-major so each chunk is contiguous/partition
        xmm = np.ascontiguousarray(
            xc.reshape(MT, P, NQ, NC).transpose(1, 2, 0, 3)
        )
        # su = -SL(G)^T @ u[ms]^T : the u-correction to cT, [R, SH] f32
        su = (
            sun.T @ np.asarray(u[b], np.float64)[ms].T
        ).astype(np.float32)
        ws = np.concatenate([su, wzt], axis=1)            # [R, SH+R] f32
        in_maps.append({"xv": xvv, "xm": xmm, "ws": ws})
    return in_maps


def _host_phase2(u_new, ct2, v):
    """Finish the v update: tiny 16x16 triangular solve per batch, f64.

    G2 uses the same fp8-quantized u_new the device used for ct2, so the
    phase-2 solve is exact coordinate descent w.r.t. that u.
    """
    f8 = ml_dtypes.float8_e4m3
    v_new = np.empty((B, N, R), np.float32)
    for b in range(B):
        un = (
            (u_new[b] * np.float32(UNSCALE)).astype(f8).astype(np.float64)
            / UNSCALE
        )
        G2 = un.T @ un
        c2 = ct2[b].T.astype(np.float64) / UNSCALE        # [N, R] = x^T @ u_new
        Y2 = c2 + EPS - np.asarray(v[b], np.float64) @ np.tril(G2, -1)
        L2 = np.triu(G2, 1) + np.diag(np.diag(G2) + EPS)
        v_new[b] = np.linalg.solve(L2.T, Y2.T).T.astype(np.float32)
    return v_new


def run(x, u, v, trace=False, trace_cores=None):
    if "nc" not in _CACHE:
        _CACHE["nc"] = _build_nc()
    nc = _CACHE["nc"]
    in_maps = _prep_in_maps(x, u, v)
    kw = {}
    if trace_cores is not None:
        kw["trace_cores"] = trace_cores
    res = run_bass_kernel_spmd(
        nc, in_maps, core_ids=list(range(NCORES)), trace=trace, **kw
    )
    u_new = np.empty((B, M, R), np.float32)
    ct2 = np.zeros((B, R, N), np.float64)
    for c in range(NCORES):
        b, j = divmod(c, NJ)
        ms = slice(j * SH, (j + 1) * SH)
        u_new[b, ms] = (
            np.asarray(res.results[c]["ou"]).transpose(1, 0, 2).reshape(SH, R)
        )
        ct2[b] += np.asarray(res.results[c]["oc"])
    v_new = _host_phase2(u_new, ct2, v)
    return (u_new, v_new), res


def kernel(x, u, v):
    (u_new, v_new), _ = run(x, u, v, trace=bool(os.environ.get("CD_TRACE")))
    return (u_new, v_new)


# revision 42
# speedup vs baseline: 1.0294x; 1.0227x over previous
"""Trainium2 Bass kernel for nn_CoordinateDescent (B=2, M=N=2048, R=16).

Math: the coordinate-descent residual e never needs materializing. With
G = v^T v and c = x @ v, the per-rank recurrence collapses to a 16x16
triangular solve per row:  a @ L = y,  L = (D+eps) + strict_upper(G),
y = c + eps - u @ strict_lower(G);  transposed  aT = W' cT' with
W' = (I+Z)^-1 diag(rd), Z = rd .* strict_lower(G), rd = 1/(diag(G)+eps),
cT' = cT - strict_lower(G)^T u^T.  (The +eps terms inside y contribute
~1e-8 absolute to a — far below fp8 rounding — and are folded into the
host-side f64 finishing step instead.)

Sharding: 8 cores = batch (2) x M-chunk (4). Core (b, j) owns 512 rows of
x[b]: it computes cT for its chunk, applies the solve to get u_new[ms_j]
on device, then computes the phase-2 partial ct2_j = u_new[ms_j]^T @
x[ms_j, :] over ALL n. The cross-chunk combine is linear, so the host sums
the 4 partials per batch and finishes the tiny 16x16 phase-2 triangular
solve in f64. No collectives (ncfw rendezvous dead time ~100us).

Numerics: x, v, and the phase-2 u_new operand ride in fp8 e4m3 (measured
end-to-end rel err: u ~4.4e-3, v ~6.9e-3 vs the 2e-2 gate; dot-product
quantization noise averages out over the 2048-long contraction). This
halves HBM traffic vs bf16 AND halves matmul count via
MatmulPerfMode.DoubleRow (two 128-deep k-subtiles per pass; warm
pipelined rate measured 216ns per 512-col pair). The 16x16 solve runs in
bf16 (f32 matmuls lower to 2 LDWEIGHTS+2 MATMUL passes — measured 1.9us
vs 1.0us for bf16). u_new is scaled by 16 before the fp8 downcast so
phase-2 operands sit in e4m3's normal range; the host divides ct2 back.
ct2 partials leave as bf16 (8-bit mantissa, v err unchanged).

Schedule: all DMAs ride the sync hardware-DGE queue in FIFO order (each
dma_start costs ~625ns of SP issue time and the completion semaphore
propagates ~900ns after the transfer). DMA descriptor size = the
contiguous per-partition run, and per-descriptor engine overhead
(~165ns) dominates small descriptors: 2KB descriptors measured only
~150GB/s/core vs ~300+ at 8KB, and ANY extra 128-descriptor dma_start
costs ~1us of stream time. So xv and the x-row copy each stream as ONE
dma (8448B / 8192B per-partition descriptors); every finer split
benchmarked slower. The PE clock is HAM-gated (1.2 GHz
cold, 2.4 GHz after a ~3.4us sustained-busy window, re-throttling after
a mostly-idle window): dummy matmuls on scratch data fill the stream
lead-in and every inter-phase gap, which halves all matmul times and is
worth ~3us end-to-end. Phase-2 writes five psum chunks, the last a
128-col stub whose tiny copy+DMA forms the final off-stream chain;
psum->SBUF copies split across Vector+Scalar (PSUM reads on 16
partitions are the tail bottleneck). Kernel is HBM-stream-bound:
~2.2MB/core at the 8-core shared-HBM aggregate (~2.3TB/s) plus ~9.4us
of fixed NRT postamble/exit (sync_barrier + 51-sem reset per engine +
dma_rearm) and ~1.3us of in-window preamble.
"""

import os
import numpy as np
import ml_dtypes

import concourse.bass as bass
import concourse.mybir as mybir
import concourse.tile as tile
from concourse import bacc
from concourse.bass_utils import run_bass_kernel_spmd

B, M, N, R = 2, 2048, 2048, 16
NCORES = 8
NJ = 4            # m-chunks per batch
SH = M // NJ      # 512
P = 128
KO = N // P       # 16 n-tiles of 128 (phase-1 contraction)
KO2 = KO // 2     # 8 DoubleRow pair groups
MT = SH // P      # 4 m-tiles of 128 in a chunk
NQ = 4            # 512-wide n-quarters for phase-2 psum
NC = N // NQ      # 512
UNSCALE = 16.0    # u_new fp8 pre-scale (max|u_new*16| << 240)
EPS = 1e-8

F32 = mybir.dt.float32
BF16 = mybir.dt.bfloat16
F8 = mybir.dt.float8e4
ALU = mybir.AluOpType
DR = mybir.MatmulPerfMode.DoubleRow

_CACHE = {}


def _build_nc():
    nc = bacc.Bacc(
        "TRN2",
        target_bir_lowering=False,
        debug=False,
        num_devices=NCORES,
    )

    # xt and vb interleave per ko-tile in one tensor: cols 0:SH are the x^T
    # block, SH:SH+R the matching v tile — one DMA stream, no handoff gap
    xv_d = nc.dram_tensor("xv", [P, KO, SH + R], F8, kind="ExternalInput")
    # q-major layout: each n-quarter chunk is 2KB contiguous per partition
    xm_d = nc.dram_tensor("xm", [P, NQ, MT, NC], F8, kind="ExternalInput")
    ws_d = nc.dram_tensor("ws", [R, SH + R], F32, kind="ExternalInput")    # su | W'^T
    ou_d = nc.dram_tensor("ou", [P, MT, R], F32, kind="ExternalOutput")    # u_new chunk
    oc_d = nc.dram_tensor("oc", [R, N], BF16, kind="ExternalOutput")       # ct2 partial

    with tile.TileContext(nc, num_cores=NCORES) as tc:
        with (
            tc.tile_pool(name="big", bufs=1) as big,
            tc.tile_pool(name="cst", bufs=1) as cst,
            tc.tile_pool(name="cps", bufs=1, space="PSUM") as cps,
            tc.tile_pool(name="ops", bufs=1, space="PSUM") as ops,
            tc.tile_pool(name="c2p", bufs=1, space="PSUM") as c2p,
        ):
            ws = cst.tile([R, SH + R], F32, tag="ws")
            wzb = cst.tile([R, R], BF16, tag="wzb")
            yb = cst.tile([R, SH], BF16, tag="yb")
            un = cst.tile([P, MT, R], F8, tag="un")
            ounat = cst.tile([P, MT, R], F32, tag="ounat")
            oc = cst.tile([R, N], BF16, tag="oc")
            ds = cst.tile([P, NC], BF16, tag="ds")      # warm-up scratch

            xv = big.tile([P, KO, SH + R], F8, tag="xv")
            xm = big.tile([P, NQ, MT, NC], F8, tag="xm")

            # The PE clock is HAM-gated: 1.2 GHz cold, 2.4 GHz after ~3.4us of
            # sustained busy (free-running 4096-cycle window). Real matmuls
            # here are DMA-paced with idle gaps, so without help the PE never
            # warms and every matmul runs at half speed. Dummy matmuls on
            # scratch data fill the stream lead-in and inter-phase gaps to
            # assert and hold the busy window. They sit in the in-order PE
            # queue, so counts are tuned to the measured gap lengths.
            # DVE memset: unblocks the burst ~0.5us earlier than gpsimd's.
            nc.vector.memset(ds[:], 0.0)
            dp = c2p.tile([R, NC], F32, tag="dp", name="dp")

            def warm(k, n=NC):
                for _ in range(k):
                    nc.tensor.matmul(dp[:, 0:n], ds[:, 0:R], ds[:, 0:n])

            # xv leads the stream as one DMA (8448B/partition descriptors —
            # bigger descriptors beat finer pacing: a 2-chunk split measured
            # ~2us slower end-to-end); ws rides second (needed at solve time).
            nc.sync.dma_start(xv[:], xv_d[:])
            nc.sync.dma_start(ws[:], ws_d[:])
            nc.sync.dma_start(xm[:], xm_d[:])
            # downcast W'^T once for the bf16 solve; depends only on ws
            nc.vector.tensor_copy(out=wzb[:], in_=ws[:, SH : SH + R])

            # warm-up burst: >=3.4us of back-to-back PE work during the DMA
            # stream lead-in flips HAM to 8/8 before the first real matmul;
            # the trailing short fills keep the free-running window
            # saturated until the xv data lands. Once warm, only a ~3.4us
            # idle window re-throttles, so small inter-phase gaps are safe.
            warm(6)
            warm(14, 128)

            # ---- phase 1: cT for the m-chunk, then the collapsed solve ----
            # 8 DoubleRow matmuls, each contracting two 128-deep n-tiles.
            ct = cps.tile([R, SH], F32, tag="cps")
            for j in range(KO2):
                nc.tensor.matmul(
                    ct[:],
                    xv[:, 2 * j : 2 * j + 2, SH : SH + R],
                    xv[:, 2 * j : 2 * j + 2, 0:SH],
                    start=(j == 0), stop=(j == KO2 - 1),
                    perf_mode=DR,
                )
            warm(3, 128)           # fill the TT0 window

            # The solve chain is split by m-halves so phase 2 starts as soon
            # as the first two u_new tiles exist: TT half -> 2 solve matmuls
            # -> fp8 downcast -> the half's phase-2 (start) matmuls, with the
            # second half's solve interleaved into the PE queue behind them.
            HS = SH // 2
            nc.vector.tensor_tensor(yb[:, 0:HS], ct[:, 0:HS], ws[:, 0:HS], ALU.add)
            nc.vector.tensor_tensor(yb[:, HS:SH], ct[:, HS:SH], ws[:, HS:SH], ALU.add)
            oup = ops.tile([P, MT * R], F32, tag="oup")
            c2 = [
                c2p.tile([R, NC], F32, tag=f"c2_{q}", name=f"c2_{q}")
                for q in range(NQ)
            ]
            # phase-2 n-chunks (q index, col offset in q, width): four full
            # psum banks, then a 128-col stub whose tiny copy+DMA forms the
            # final off-stream chain. The stub shares q3's data but its own
            # psum tile (in the ops pool next to oup).
            stub = ops.tile([R, P], F32, tag="stub")
            CH2 = [
                (c2[0], 0, 0, NC), (c2[1], 1, 0, NC), (c2[2], 2, 0, NC),
                (c2[3], 3, 0, NC - P), (stub, 3, NC - P, P),
            ]

            def p2mm(i, t2, start, stop):
                tile_, q, qo, w = CH2[i]
                nc.tensor.matmul(
                    tile_[:, 0:w],
                    un[:, 2 * t2 : 2 * t2 + 2, :],
                    xm[:, q, 2 * t2 : 2 * t2 + 2, qo : qo + w],
                    start=start, stop=stop, perf_mode=DR,
                )

            for t in (0, 1):
                nc.tensor.matmul(
                    oup[:, t * R : (t + 1) * R], yb[:, t * P : (t + 1) * P], wzb[:]
                )
            nc.scalar.activation(
                un[:, 0:2].rearrange("p t r -> p (t r)"), oup[:, 0 : 2 * R],
                mybir.ActivationFunctionType.Copy,
                scale=UNSCALE,
            )
            warm(4, 128)           # fill the un downcast window
            for i in (0, 1):
                p2mm(i, 0, True, False)
            warm(2, 128)           # keep the HAM window busy through the
            for t in (2, 3):
                nc.tensor.matmul(
                    oup[:, t * R : (t + 1) * R], yb[:, t * P : (t + 1) * P], wzb[:]
                )
            nc.scalar.activation(
                un[:, 2:4].rearrange("p t r -> p (t r)"), oup[:, 2 * R : 4 * R],
                mybir.ActivationFunctionType.Copy,
                scale=UNSCALE,
            )
            warm(3, 128)           # solve/downcast chain (a mid-kernel
            p2mm(2, 0, True, False)  # re-throttle costs ~2-3us); chunk-2's
            # open matmul covers the ACT1->PE semaphore latency
            nc.vector.tensor_copy(
                out=ounat[:].rearrange("p t r -> p (t r)"), in_=oup[:]
            )
            nc.sync.dma_start(ou_d[:], ounat[:])

            # ---- phase 2 close: each chunk closes as early as possible so
            # its psum->SBUF copy starts draining immediately (copies are
            # PSUM-read bound on 16 partitions and the two copy engines are
            # the tail bottleneck; splitting each chunk across Vector+Scalar
            # halves the latency; bf16 out halves the output DMA bytes —
            # partials are 8-bit-mantissa safe, measured v err unchanged).
            def close_and_copy(i):
                tile_, q, qo, w = CH2[i]
                o = q * NC + qo
                p2mm(i, 1, False, True)
                if w > P:
                    # V measures ~1.60ns/col, S ~1.78ns/col: split 55/45 so
                    # both copy engines finish together
                    h = (w * 11 // 20) // 8 * 8
                    nc.vector.tensor_copy(
                        out=oc[:, o : o + h], in_=tile_[:, 0:h]
                    )
                    nc.scalar.activation(
                        oc[:, o + h : o + w], tile_[:, h:w],
                        mybir.ActivationFunctionType.Copy,
                    )
                else:
                    nc.vector.tensor_copy(out=oc[:, o : o + w], in_=tile_[:, 0:w])

            close_and_copy(0)
            close_and_copy(1)
            nc.sync.dma_start(oc_d[:, 0 : N // 2], oc[:, 0 : N // 2])
            close_and_copy(2)
            p2mm(3, 0, True, False)
            close_and_copy(3)
            p2mm(4, 0, True, False)
            close_and_copy(4)
            nc.sync.dma_start(oc_d[:, N // 2 : N], oc[:, N // 2 : N])

    nc.compile()
    return nc


def _host_solver_inputs(v):
    """Per batch: -strict_lower(G) and W'^T, W' = inv(I+Z) diag(rd), G = v^T v."""
    out = []
    for b in range(B):
        vb = np.asarray(v[b], np.float64)
        G = vb.T @ vb
        rd = 1.0 / (np.diag(G) + EPS)
        sun = -np.tril(G, -1)
        Z = rd[:, None] * np.tril(G, -1)
        W = np.linalg.inv(np.eye(R) + Z) * rd[None, :]  # (I+Z)^-1 then col-scale
        out.append((sun, np.ascontiguousarray(W.T).astype(np.float32)))
    return out


def _prep_in_maps(x, u, v):
    f8 = ml_dtypes.float8_e4m3
    solver = _host_solver_inputs(v)
    per_batch = []
    for b in range(B):
        xq = np.asarray(x[b], np.float32).astype(f8)      # quantize once
        vq = np.asarray(v[b], np.float32).astype(f8)
        vbb = vq.reshape(KO, P, R).swapaxes(0, 1)         # [P, KO, R]
        per_batch.append((xq, vbb))
    in_maps = []
    for c in range(NCORES):
        b, j = divmod(c, NJ)
        xq, vbb = per_batch[b]
        sun, wzt = solver[b]
        ms = slice(j * SH, (j + 1) * SH)
        xc = xq[ms]  # [SH, N] fp8
        xt = np.ascontiguousarray(xc.T).reshape(KO, P, SH).swapaxes(0, 1)
        xvv = np.concatenate([xt, vbb], axis=2)           # [P, KO, SH+R] fp8
        # [P, NQ, MT, NC]: q-major so each chunk is contiguous/partition
        xmm = np.ascontiguousarray(
            xc.reshape(MT, P, NQ, NC).transpose(1, 2, 0, 3)
        )
        # su = -SL(G)^T @ u[ms]^T : the u-correction to cT, [R, SH] f32
        su = (
            sun.T @ np.asarray(u[b], np.float64)[ms].T
        ).astype(np.float32)
        ws = np.concatenate([su, wzt], axis=1)            # [R, SH+R] f32
        in_maps.append({"xv": xvv, "xm": xmm, "ws": ws})
    return in_maps


def _host_phase2(u_new, ct2, v):
    """Finish the v update: tiny 16x16 triangular solve per batch, f64.

    G2 uses the same fp8-quantized u_new the device used for ct2, so the
    phase-2 solve is exact coordinate descent w.r.t. that u.
    """
    f8 = ml_dtypes.float8_e4m3
    v_new = np.empty((B, N, R), np.float32)
    for b in range(B):
        un = (
            (u_new[b] * np.float32(UNSCALE)).astype(f8).astype(np.float64)
            / UNSCALE
        )
        G2 = un.T @ un
        c2 = ct2[b].T.astype(np.float64) / UNSCALE        # [N, R] = x^T @ u_new
        Y2 = c2 + EPS - np.asarray(v[b], np.float64) @ np.tril(G2, -1)
        L2 = np.triu(G2, 1) + np.diag(np.diag(G2) + EPS)
        v_new[b] = np.linalg.solve(L2.T, Y2.T).T.astype(np.float32)
    return v_new


def run(x, u, v, trace=False, trace_cores=None):
    if "nc" not in _CACHE:
        _CACHE["nc"] = _build_nc()
    nc = _CACHE["nc"]
    in_maps = _prep_in_maps(x, u, v)
    kw = {}
    if trace_cores is not None:
        kw["trace_cores"] = trace_cores
    res = run_bass_kernel_spmd(
        nc, in_maps, core_ids=list(range(NCORES)), trace=trace, **kw
    )
    u_new = np.empty((B, M, R), np.float32)
    ct2 = np.zeros((B, R, N), np.float64)
    for c in range(NCORES):
        b, j = divmod(c, NJ)
        ms = slice(j * SH, (j + 1) * SH)
        u_new[b, ms] = (
            np.asarray(res.results[c]["ou"]).transpose(1, 0, 2).reshape(SH, R)
        )
        ct2[b] += np.asarray(res.results[c]["oc"])
    v_new = _host_phase2(u_new, ct2, v)
    return (u_new, v_new), res


def kernel(x, u, v):
    (u_new, v_new), _ = run(x, u, v, trace=bool(os.environ.get("CD_TRACE")))
    return (u_new, v_new)
